# revision 2
# baseline (speedup 1.0000x reference)
"""MoE expert-network kernel for 8 Trainium2 NeuronCores.

Strategy: expert parallelism (E == n_cores == 8). The host dispatches each
token to its expert's core (an all-to-all in numpy), folds the inference-mode
BatchNorm into the expert weights/bias, and each core runs one dense
[cap, 512] @ [512, 512] GEMM fused with bias + SiLU via the activation engine.

All device tensors are laid out host-side as the exact SBUF tile images
(128-partition-major, block-contiguous per token tile) so every DMA is a
plain 2D contiguous copy with multi-KB lines.

Per-core device program (identical on all cores, SPMD):
  inputs : xs  [128, KC*cap]        fp16 - token tiles, partition-major blocks
           wx0 [128, KC*128+KC*s0]  fp16 - BN-folded weights for m=0 ++ the
                                         first x tile (one leading DMA)
           wm  [128, 3*KC*128]      fp16 - weights for m=1..3 (3 racing DMAs)
           bs  [128, MC]            fp32 - BN-folded bias tile image
  output : os  [128, MC*cap]        fp16 - silu(x @ W + b), (tile, m)-major
x is shipped fp16 (~2e-4 rel error, halves the dominant stream); the host
scatters the result back into the full [B, 512] fp32 output.

Pipeline design notes (from perfetto traces):
  - exec time is measured first-user-instruction -> last-instruction, and the
    epilogue includes a fixed ~5us walrus semaphore sweep, so the win is all
    in starting the PE early and finishing the drain early;
  - the weight image is split per output-feature block m: [w_m0|x_tile0]
    rides first on the sync HWDGE ring, then w_m1..w_m3 as separate DMAs.
    The PE's m-loop for the first token tile races the weight stream, so
    real matmuls start ~3us in instead of waiting ~5.5us for one fused
    655KB image (concurrent DMAs round-robin the engines, so a monolithic
    transfer's tail is starved by the x stream queued behind it);
  - x tiles 1-2 ride the scalar HWDGE ring (idle until the first SILU),
    later tiles the sync ring behind the weight blocks;
  - a short burst of 128-col dummy matmuls on a zeroed tile bridges the
    ~3us weight lead-in so the PE's HAM clock-gate window (~3.4us of
    sustained activity -> 2.4GHz) is mostly paid before real work arrives;
  - no dummy SILU: the ACT table loads (~2.6us) auto-insert on the scalar
    queue after the x1/x2 triggers and before the first real SILU, which
    only delays tile0's activations, never the matmul stream;
  - psum tiles span 2 banks: one SILU instruction reads up to 1024
    columns, halving the ACT engine's per-op overhead;
  - stores are per (m-pair, tile) slices fired right after each SILU on the
    GpSimd SWDGE ring; the last tile's stores ride per-m, with the terminal
    one on the idle sync HWDGE ring for low completion latency.
"""

import sys

for _p in ("/opt/trn_rl_repo",):
    if _p not in sys.path:
        sys.path.append(_p)

import numpy as np

import concourse.bass as bass
import concourse.mybir as mybir
import concourse.tile as tile
from concourse import bacc
from concourse.bass_utils import run_bass_kernel_spmd

B = 32768
IN = 512
HID = 512
E = 8
NCORES = 8
EPS = 1e-5
P = 128  # SBUF partitions
NT = 512  # matmul moving-dim chunk (one fp32 PSUM bank)

KC = IN // P  # contraction chunks
MC = HID // P  # output-feature chunks
W1 = KC * P  # weight-image columns per output block m
NWARM = 14  # HAM-prewarm dummy matmuls (N=128, cold ~150ns each); they
# bridge the PE from ~0.9us until the w_m0 + x tile0 DMA lands (~3us) and
# pre-pay most of the ~3.4us HAM cold window.


def plan_sizes(cap: int) -> list:
    """Token-tile sizes: tiny tiles at the start (fast pipeline ramp: first
    matmul can begin after only a 128-token DMA) and a 512 tail tile (short
    final ACT->store chain), 1024-wide tiles in the middle."""
    if cap < 1280:  # not reachable for the real token distribution
        return [min(512, cap - o) for o in range(0, cap, 512)]
    sizes = [128, 256, 512]
    # Reserve a 512-token tail tile: its per-m SILUs (~720ns) are SHORTER
    # than their matmul groups (~852ns), so the ACT engine tracks the PE
    # through the final tile instead of queueing 4 small SILUs after the
    # last matmul (a 256 tail measured ~2us of serial ACT tail).
    rem = cap - 896 - 512
    while rem >= 1024:
        sizes.append(1024)
        rem -= 1024
    if rem:
        sizes.append(rem)
    sizes.append(512)
    return sizes


def build_bass(cap: int, act: str = "silu") -> bass.Bass:
    nc = bacc.Bacc(
        "TRN2",
        target_bir_lowering=False,
        debug=False,
        enable_asserts=False,
        num_devices=NCORES,
    )
    f32 = mybir.dt.float32
    f16 = mybir.dt.float16

    tiles = []
    n0 = 0
    for s in plan_sizes(cap):
        tiles.append((n0, s))
        n0 += s
    s0 = tiles[0][1]

    xs = nc.dram_tensor("xs", [P, KC * cap], f16, kind="ExternalInput").ap()
    wx0 = nc.dram_tensor("wx0", [P, W1 + KC * s0], f16, kind="ExternalInput").ap()
    wm = nc.dram_tensor("wm", [P, (MC - 1) * W1], f16, kind="ExternalInput").ap()
    bs = nc.dram_tensor("bs", [P, MC], f32, kind="ExternalInput").ap()
    os_ = nc.dram_tensor("os", [P, MC * cap], f16, kind="ExternalOutput").ap()

    with tile.TileContext(nc) as tc:
        with (
            tc.tile_pool(name="wpool", bufs=1) as wpool,
            tc.tile_pool(name="xpool", bufs=4) as xpool,
            tc.tile_pool(name="opool", bufs=3) as opool,
            tc.tile_pool(name="pp", bufs=4, space="PSUM") as pp,
        ):
            # Dummy matmuls on a small zeroed tile start the HAM clock-gate
            # warmup immediately; the PE is busy from ~0.9us while the
            # weight blocks are still in flight.
            warm = wpool.tile([P, P], f16, tag="warm", name="warm")
            nc.gpsimd.memset(warm, 0.0)

            # Weight stream on the sync HWDGE ring: [w_m0|x_tile0] first,
            # then w_m1..w_m3 as separate completions so the PE's m-loop on
            # the first token tile can race the stream.
            wxt = wpool.tile([P, W1 + KC * s0], f16, tag="wx0", name="wx0")
            nc.sync.dma_start(out=wxt, in_=wx0)
            wts = []
            for m in range(1, MC):
                wt = wpool.tile([P, W1], f16, tag=f"wm{m}", name=f"wm{m}")
                nc.sync.dma_start(out=wt, in_=wm[:, (m - 1) * W1 : m * W1])
                wts.append(wt)

            # Bias + early x tiles on the scalar HWDGE ring (idle until the
            # first SILU; the ACT table loads auto-insert after these).
            bt = wpool.tile([P, MC], f32, tag="bt", name="bt")
            nc.scalar.dma_start(out=bt, in_=bs)

            wps = pp.tile([P, NT], f32, tag="ps", name="wps")
            for _ in range(NWARM):
                nc.tensor.matmul(wps[:, :P], lhsT=warm, rhs=warm, start=True, stop=True)

            for ti, (n0, nt) in enumerate(tiles):
                if ti == 0:
                    xt = None  # tile 1 lives inside the fused wx0 image
                else:
                    xt = xpool.tile([P, KC, nt], f16, tag="xt", name="xt")
                    eng = nc.scalar if ti <= 2 else nc.sync
                    eng.dma_start(out=xt, in_=xs[:, KC * n0 : KC * (n0 + nt)])
                ot = opool.tile([P, MC, nt], f16, tag="ot", name="ot")
                ng = -(-nt // NT)  # 512-chunks in this tile (<= 2)
                for m in range(MC):
                    # ng PSUM banks; one SILU reads the whole [P, nt] span
                    ps = pp.tile([P, ng * NT], f32, tag="ps", name="ps")
                    for g in range(ng):
                        off = g * NT
                        ns = min(NT, nt - off)
                        for k in range(KC):
                            rhs = (
                                wxt[:, W1 + k * s0 + off : W1 + k * s0 + off + ns]
                                if ti == 0
                                else xt[:, k, off : off + ns]
                            )
                            lhsT = (
                                wxt[:, k * P : (k + 1) * P]
                                if m == 0
                                else wts[m - 1][:, k * P : (k + 1) * P]
                            )
                            nc.tensor.matmul(
                                ps[:, off : off + ns],
                                lhsT=lhsT,
                                rhs=rhs,
                                start=(k == 0),
                                stop=(k == KC - 1),
                            )
                    osl = ot[:, m]
                    pview = ps[:, :nt]
                    if act == "silu":
                        nc.scalar.activation(
                            osl,
                            pview,
                            mybir.ActivationFunctionType.Silu,
                            bias=bt[:, m : m + 1],
                        )
                    else:
                        # CoreSim has no Silu: Identity+Sigmoid+mul
                        yt = opool.tile([P, nt], f32, tag="yt", name="yt")
                        nc.scalar.activation(
                            yt,
                            pview,
                            mybir.ActivationFunctionType.Identity,
                            bias=bt[:, m : m + 1],
                        )
                        st = opool.tile([P, nt], f32, tag="st", name="st")
                        nc.scalar.activation(
                            st,
                            pview,
                            mybir.ActivationFunctionType.Sigmoid,
                            bias=bt[:, m : m + 1],
                        )
                        nc.vector.tensor_mul(osl, yt, st)
                    # Store m-pairs (after the m=1 / m=3 SILUs): smooth
                    # out-stream on the (otherwise idle) GpSimd SWDGE ring
                    # without flooding the Q7 descriptor queue (~1us
                    # emission per store op). The last tile stores per-m,
                    # with the terminal transfer (the one the exit drain
                    # waits on) on the idle sync HWDGE ring for low
                    # completion latency.
                    if ti == len(tiles) - 1:
                        out_eng = nc.sync if m == MC - 1 else nc.gpsimd
                        out_eng.dma_start(
                            out=os_[:, MC * n0 + m * nt : MC * n0 + (m + 1) * nt],
                            in_=osl,
                        )
                    elif m % 2 == 1:
                        nc.gpsimd.dma_start(
                            out=os_[
                                :, MC * n0 + (m - 1) * nt : MC * n0 + (m + 1) * nt
                            ],
                            in_=ot[:, m - 1 : m + 1],
                        )

    nc.compile()
    return nc


def prepare(inputs: dict) -> tuple:
    x = np.ascontiguousarray(np.asarray(inputs["x"], dtype=np.float32))
    idx = np.asarray(inputs["expert_indices"]).astype(np.int64)
    ew = np.asarray(inputs["expert_weights"], dtype=np.float32)
    eb = np.asarray(inputs["expert_biases"], dtype=np.float32)
    gw = np.asarray(inputs["bn_weights"], dtype=np.float32)
    gb = np.asarray(inputs["bn_biases"], dtype=np.float32)
    rm = np.asarray(inputs["running_mean"], dtype=np.float32)
    rv = np.asarray(inputs["running_var"], dtype=np.float32)

    # Fold inference BN into the expert weight/bias:
    #   y = (x @ W + eb - rm) * gw/sqrt(rv+eps) + gb = x @ (W*s) + (eb-rm)*s + gb
    s = gw / np.sqrt(rv + EPS)
    wf = ew * s[:, None, :]
    bf = (eb - rm) * s + gb

    perms = [np.nonzero(idx == e)[0] for e in range(E)]
    counts = [len(p) for p in perms]
    cap = max(512, -(-max(counts) // P) * P)
    tiles = []
    n0 = 0
    for t in plan_sizes(cap):
        tiles.append((n0, t))
        n0 += t
    s0 = tiles[0][1]

    in_maps = []
    for e in range(E):
        xT = np.zeros((IN, cap), dtype=np.float16)
        if counts[e]:
            xT[:, : counts[e]] = x[perms[e]].T.astype(np.float16)
        xv = xT.reshape(KC, P, cap)
        xs = np.empty((P, KC * cap), dtype=np.float16)
        for n0, nt in tiles:
            xs[:, KC * n0 : KC * (n0 + nt)] = (
                xv[:, :, n0 : n0 + nt].transpose(1, 0, 2).reshape(P, KC * nt)
            )
        # m-major weight image: ws[p, ((m*KC + k)*P + j)] = W[k*P + p, m*P + j]
        ws = (
            wf[e]
            .astype(np.float16)
            .reshape(KC, P, MC, P)
            .transpose(1, 2, 0, 3)
            .reshape(P, MC * KC * P)
        )
        # leading image: m=0 weights ++ first x tile; the rest per-m
        wx0 = np.concatenate([ws[:, :W1], xs[:, : KC * s0]], axis=1)
        wm = ws[:, W1:]
        bs = np.ascontiguousarray(bf[e].reshape(MC, P).T)
        in_maps.append(
            {
                "xs": xs,
                "wx0": np.ascontiguousarray(wx0),
                "wm": np.ascontiguousarray(wm),
                "bs": bs,
            }
        )
    return cap, tiles, perms, counts, in_maps


def combine(results: list, cap, tiles, perms, counts) -> np.ndarray:
    out = np.empty((B, HID), dtype=np.float32)
    for e in range(E):
        if not counts[e]:
            continue
        ob = results[e]["os"]
        oT = np.empty((HID, cap), dtype=np.float32)
        for n0, nt in tiles:
            # per-(tile, m) blocks: [P, nt] at column MC*n0 + m*nt
            oT[:, n0 : n0 + nt] = (
                ob[:, MC * n0 : MC * (n0 + nt)]
                .reshape(P, MC, nt)
                .transpose(1, 0, 2)
                .reshape(HID, nt)
            )
        out[perms[e]] = oT[:, : counts[e]].T
    return out


def kernel(**inputs) -> np.ndarray:
    cap, tiles, perms, counts, in_maps = prepare(inputs)
    nc = build_bass(cap)
    res = run_bass_kernel_spmd(nc, in_maps, core_ids=list(range(NCORES)))
    return combine(res.results, cap, tiles, perms, counts)


# revision 5
# speedup vs baseline: 1.0340x; 1.0340x over previous
"""MoE expert-network kernel for 8 Trainium2 NeuronCores.

Strategy: expert parallelism (E == n_cores == 8). The host dispatches each
token to its expert's core (an all-to-all in numpy), folds the inference-mode
BatchNorm into the expert weights/bias, and each core runs one dense
[cap, 512] @ [512, 512] GEMM fused with bias + SiLU via the activation engine.

All device tensors are laid out host-side as the exact SBUF tile images
(128-partition-major, block-contiguous per token tile) so every DMA is a
plain 2D contiguous copy with multi-KB lines.

Per-core device program (identical on all cores, SPMD):
  inputs : xs  [128, KC*cap]        fp16 - token tiles, partition-major blocks
           wx0 [128, KC*128+KC*s0]  fp16 - BN-folded weights for m=0 ++ the
                                         first x tile (one leading DMA)
           wm  [128, 3*KC*128]      fp16 - weights for m=1..3 (3 racing DMAs)
           bs  [128, MC]            fp32 - BN-folded bias tile image
  output : os  [128, MC*cap]        fp16 - silu(x @ W + b), (tile, m)-major
x is shipped fp16 (~2e-4 rel error, halves the dominant stream); the host
scatters the result back into the full [B, 512] fp32 output.

Pipeline design notes (from perfetto traces):
  - exec time is measured first-user-instruction -> last-instruction, and the
    epilogue includes a fixed ~5us walrus semaphore sweep, so the win is all
    in starting the PE early and finishing the drain early;
  - the weight image is split per output-feature block m: [w_m0|x_tile0]
    rides first on the sync HWDGE ring, then w_m1..w_m3 as separate DMAs.
    The PE's m-loop for the first token tile races the weight stream, so
    real matmuls start ~3us in instead of waiting ~5.5us for one fused
    655KB image (concurrent DMAs round-robin the engines, so a monolithic
    transfer's tail is starved by the x stream queued behind it);
  - x tiles 1-2 ride the scalar HWDGE ring (idle until the first SILU),
    later tiles the sync ring behind the weight blocks;
  - a short burst of 128-col dummy matmuls on a zeroed tile bridges the
    ~3us weight lead-in so the PE's HAM clock-gate window (~3.4us of
    sustained activity -> 2.4GHz) is mostly paid before real work arrives;
  - no dummy SILU: the ACT table loads (~2.6us) auto-insert on the scalar
    queue after the x1/x2 triggers and before the first real SILU, which
    only delays tile0's activations, never the matmul stream;
  - psum tiles span 2 banks: one SILU instruction reads up to 1024
    columns, halving the ACT engine's per-op overhead;
  - stores are per (m-pair, tile) slices fired right after each SILU on the
    GpSimd SWDGE ring; the last tile's stores ride per-m, with the terminal
    one on the idle sync HWDGE ring for low completion latency.
"""

import sys

for _p in ("/opt/trn_rl_repo",):
    if _p not in sys.path:
        sys.path.append(_p)

import numpy as np

import concourse.bass as bass
import concourse.mybir as mybir
import concourse.tile as tile
from concourse import bacc
from concourse.bass_utils import run_bass_kernel_spmd

B = 32768
IN = 512
HID = 512
E = 8
NCORES = 8
EPS = 1e-5
P = 128  # SBUF partitions
NT = 512  # matmul moving-dim chunk (one fp32 PSUM bank)

KC = IN // P  # contraction chunks
MC = HID // P  # output-feature chunks
W1 = KC * P  # weight-image columns per output block m
NWARM = 18  # HAM-prewarm dummy matmuls (N=128, cold ~107ns each); they
# bridge the PE from ~1.5us until the w_m0 + x tile0 DMA lands (~3.2us) and
# pre-pay part of the ~3.4us HAM cold window.


def plan_sizes(cap: int) -> list:
    """Token-tile sizes: tiny tiles at the start (fast pipeline ramp: first
    matmul can begin after only a 128-token DMA) and a 512 tail tile (short
    final ACT->store chain), 1024-wide tiles in the middle."""
    if cap < 1280:  # not reachable for the real token distribution
        return [min(512, cap - o) for o in range(0, cap, 512)]
    sizes = [128, 256, 512]
    # Reserve a 512-token tail tile: its per-m SILUs (~720ns) are SHORTER
    # than their matmul groups (~852ns), so the ACT engine tracks the PE
    # through the final tile instead of queueing 4 small SILUs after the
    # last matmul (a 256 tail measured ~2us of serial ACT tail).
    rem = cap - 896 - 512
    while rem >= 1024:
        sizes.append(1024)
        rem -= 1024
    if rem:
        sizes.append(rem)
    sizes.append(512)
    return sizes


def build_bass(cap: int, act: str = "silu") -> bass.Bass:
    nc = bacc.Bacc(
        "TRN2",
        target_bir_lowering=False,
        debug=False,
        enable_asserts=False,
        num_devices=NCORES,
    )
    f32 = mybir.dt.float32
    f16 = mybir.dt.float16

    tiles = []
    n0 = 0
    for s in plan_sizes(cap):
        tiles.append((n0, s))
        n0 += s
    s0 = tiles[0][1]

    xs = nc.dram_tensor("xs", [P, KC * cap], f16, kind="ExternalInput").ap()
    wx0 = nc.dram_tensor("wx0", [P, W1 + KC * s0], f16, kind="ExternalInput").ap()
    wm = nc.dram_tensor("wm", [P, (MC - 1) * W1], f16, kind="ExternalInput").ap()
    bs = nc.dram_tensor("bs", [P, MC], f32, kind="ExternalInput").ap()
    os_ = nc.dram_tensor("os", [P, MC * cap], f16, kind="ExternalOutput").ap()

    with tile.TileContext(nc) as tc:
        with (
            tc.tile_pool(name="wpool", bufs=1) as wpool,
            tc.tile_pool(name="xpool", bufs=4) as xpool,
            tc.tile_pool(name="opool", bufs=3) as opool,
            tc.tile_pool(name="pp", bufs=4, space="PSUM") as pp,
        ):
            # Weight stream on the sync HWDGE ring: [w_m0|x_tile0] first,
            # then w_m1..w_m3 as separate completions so the PE's m-loop on
            # the first token tile can race the stream. All x tiles queue
            # behind them on the same ring: one FIFO keeps completion order
            # predictable (concurrent queues split HBM bandwidth and starve
            # whichever tile the PE needs next).
            wxt = wpool.tile([P, W1 + KC * s0], f16, tag="wx0", name="wx0")
            nc.sync.dma_start(out=wxt, in_=wx0)
            wts = []
            for m in range(1, MC):
                wt = wpool.tile([P, W1], f16, tag=f"wm{m}", name=f"wm{m}")
                nc.sync.dma_start(out=wt, in_=wm[:, (m - 1) * W1 : m * W1])
                wts.append(wt)

            # Bias on the scalar HWDGE ring (idle until the first SILU; the
            # ACT table loads auto-insert after this trigger).
            bt = wpool.tile([P, MC], f32, tag="bt", name="bt")
            nc.scalar.dma_start(out=bt, in_=bs)

            # Dummy matmuls on a small zeroed tile bridge the weight lead-in
            # and pre-pay most of the HAM clock-gate window.
            warm = wpool.tile([P, P], f16, tag="warm", name="warm")
            nc.gpsimd.memset(warm, 0.0)
            wps = pp.tile([P, NT], f32, tag="ps", name="wps")
            for _ in range(NWARM):
                nc.tensor.matmul(wps[:, :P], lhsT=warm, rhs=warm, start=True, stop=True)

            for ti, (n0, nt) in enumerate(tiles):
                if ti == 0:
                    xt = None  # tile 1 lives inside the fused wx0 image
                else:
                    xt = xpool.tile([P, KC, nt], f16, tag="xt", name="xt")
                    nc.sync.dma_start(out=xt, in_=xs[:, KC * n0 : KC * (n0 + nt)])
                ot = opool.tile([P, MC, nt], f16, tag="ot", name="ot")
                ng = -(-nt // NT)  # 512-chunks in this tile (<= 2)
                for m in range(MC):
                    # ng PSUM banks; one SILU reads the whole [P, nt] span
                    ps = pp.tile([P, ng * NT], f32, tag="ps", name="ps")
                    for g in range(ng):
                        off = g * NT
                        ns = min(NT, nt - off)
                        for k in range(KC):
                            rhs = (
                                wxt[:, W1 + k * s0 + off : W1 + k * s0 + off + ns]
                                if ti == 0
                                else xt[:, k, off : off + ns]
                            )
                            lhsT = (
                                wxt[:, k * P : (k + 1) * P]
                                if m == 0
                                else wts[m - 1][:, k * P : (k + 1) * P]
                            )
                            nc.tensor.matmul(
                                ps[:, off : off + ns],
                                lhsT=lhsT,
                                rhs=rhs,
                                start=(k == 0),
                                stop=(k == KC - 1),
                            )
                    osl = ot[:, m]
                    pview = ps[:, :nt]
                    if act == "silu":
                        nc.scalar.activation(
                            osl,
                            pview,
                            mybir.ActivationFunctionType.Silu,
                            bias=bt[:, m : m + 1],
                        )
                    else:
                        # CoreSim has no Silu: Identity+Sigmoid+mul
                        yt = opool.tile([P, nt], f32, tag="yt", name="yt")
                        nc.scalar.activation(
                            yt,
                            pview,
                            mybir.ActivationFunctionType.Identity,
                            bias=bt[:, m : m + 1],
                        )
                        st = opool.tile([P, nt], f32, tag="st", name="st")
                        nc.scalar.activation(
                            st,
                            pview,
                            mybir.ActivationFunctionType.Sigmoid,
                            bias=bt[:, m : m + 1],
                        )
                        nc.vector.tensor_mul(osl, yt, st)
                    # Store m-pairs (after the m=1 / m=3 SILUs): smooth
                    # out-stream on the (otherwise idle) GpSimd SWDGE ring
                    # without flooding the Q7 descriptor queue (~1us
                    # emission per store op). The last tile stores per-m,
                    # with the terminal transfer (the one the exit drain
                    # waits on) on the idle sync HWDGE ring for low
                    # completion latency.
                    if ti == len(tiles) - 1:
                        out_eng = nc.sync if m == MC - 1 else nc.gpsimd
                        out_eng.dma_start(
                            out=os_[:, MC * n0 + m * nt : MC * n0 + (m + 1) * nt],
                            in_=osl,
                        )
                    elif m % 2 == 1:
                        nc.gpsimd.dma_start(
                            out=os_[
                                :, MC * n0 + (m - 1) * nt : MC * n0 + (m + 1) * nt
                            ],
                            in_=ot[:, m - 1 : m + 1],
                        )

    nc.compile()
    return nc


def prepare(inputs: dict) -> tuple:
    x = np.ascontiguousarray(np.asarray(inputs["x"], dtype=np.float32))
    idx = np.asarray(inputs["expert_indices"]).astype(np.int64)
    ew = np.asarray(inputs["expert_weights"], dtype=np.float32)
    eb = np.asarray(inputs["expert_biases"], dtype=np.float32)
    gw = np.asarray(inputs["bn_weights"], dtype=np.float32)
    gb = np.asarray(inputs["bn_biases"], dtype=np.float32)
    rm = np.asarray(inputs["running_mean"], dtype=np.float32)
    rv = np.asarray(inputs["running_var"], dtype=np.float32)

    # Fold inference BN into the expert weight/bias:
    #   y = (x @ W + eb - rm) * gw/sqrt(rv+eps) + gb = x @ (W*s) + (eb-rm)*s + gb
    s = gw / np.sqrt(rv + EPS)
    wf = ew * s[:, None, :]
    bf = (eb - rm) * s + gb

    perms = [np.nonzero(idx == e)[0] for e in range(E)]
    counts = [len(p) for p in perms]
    cap = max(512, -(-max(counts) // P) * P)
    tiles = []
    n0 = 0
    for t in plan_sizes(cap):
        tiles.append((n0, t))
        n0 += t
    s0 = tiles[0][1]

    in_maps = []
    for e in range(E):
        xT = np.zeros((IN, cap), dtype=np.float16)
        if counts[e]:
            xT[:, : counts[e]] = x[perms[e]].T.astype(np.float16)
        xv = xT.reshape(KC, P, cap)
        xs = np.empty((P, KC * cap), dtype=np.float16)
        for n0, nt in tiles:
            xs[:, KC * n0 : KC * (n0 + nt)] = (
                xv[:, :, n0 : n0 + nt].transpose(1, 0, 2).reshape(P, KC * nt)
            )
        # m-major weight image: ws[p, ((m*KC + k)*P + j)] = W[k*P + p, m*P + j]
        ws = (
            wf[e]
            .astype(np.float16)
            .reshape(KC, P, MC, P)
            .transpose(1, 2, 0, 3)
            .reshape(P, MC * KC * P)
        )
        # leading image: m=0 weights ++ first x tile; the rest per-m
        wx0 = np.concatenate([ws[:, :W1], xs[:, : KC * s0]], axis=1)
        wm = ws[:, W1:]
        bs = np.ascontiguousarray(bf[e].reshape(MC, P).T)
        in_maps.append(
            {
                "xs": xs,
                "wx0": np.ascontiguousarray(wx0),
                "wm": np.ascontiguousarray(wm),
                "bs": bs,
            }
        )
    return cap, tiles, perms, counts, in_maps


def combine(results: list, cap, tiles, perms, counts) -> np.ndarray:
    out = np.empty((B, HID), dtype=np.float32)
    for e in range(E):
        if not counts[e]:
            continue
        ob = results[e]["os"]
        oT = np.empty((HID, cap), dtype=np.float32)
        for n0, nt in tiles:
            # per-(tile, m) blocks: [P, nt] at column MC*n0 + m*nt
            oT[:, n0 : n0 + nt] = (
                ob[:, MC * n0 : MC * (n0 + nt)]
                .reshape(P, MC, nt)
                .transpose(1, 0, 2)
                .reshape(HID, nt)
            )
        out[perms[e]] = oT[:, : counts[e]].T
    return out


def kernel(**inputs) -> np.ndarray:
    cap, tiles, perms, counts, in_maps = prepare(inputs)
    nc = build_bass(cap)
    res = run_bass_kernel_spmd(nc, in_maps, core_ids=list(range(NCORES)))
    return combine(res.results, cap, tiles, perms, counts)
